# revision 38
# baseline (speedup 1.0000x reference)
"""Distributed cosine-similarity kNN retrieval (EpisodicSDM) on 8 Trainium2 cores.

Reference:
    x_n = normalize(x); k_n = normalize(keys)
    scores = x_n @ k_n.T                [B, N]
    top_vals, top_idx = top_k(scores, 8)
    out = sum_k softmax(top_vals)_k * values[top_idx_k]

Host prep (pure data movement / O(N*D) scaling, no HW time):
  - normalize x and keys in fp32; pad keys/values to NPAD rows
  - pre-transpose to bf16: xT [D, B], kT [D, NLOC] per-core shard

Dispatch A (keys sharded along N, all queries on every core):
  - scores S = xT.T @ kT in bf16 (fp32 PSUM), 26 n-tiles of 512 per
    128-query tile, grouped into 6 psA/psB block pairs + 1 leftover
  - fold1 -> m1 bf16 [128, 6656]: 3 blocks DVE-direct (ACT copies psA
    to SBUF bf16, DVE TT max vs psB in PSUM), 3 blocks ACT-extracted
    (both copies on ACT, DVE TT max in bf16 at 2x), leftover self-max
  - fold2 -> m2 bf16 [128, 3328] (DVE TT bf16 2x, halves pairing)
  - pack on GPSIMD: numeric bf16->fp32 convert (== bits<<16), then OR
    a 12-bit slot iota into the low mantissa -> tie-free packed fp32
  - DVE max8 -> per-core top-8 packed cells [B, 8]

Host glue: concat candidates [B, 64]; OR core id into bits 12:15.

Dispatch B (queries sharded, 256 per core; full key/value tables):
  - top-12 of 64 via max8 + match_replace + max8 (tie-free: low 16
    bits of every candidate are distinct)
  - decode rows arithmetically: j = v&0xFFF, core = (v>>12)&7,
    i in {j, j+3328}; key0 = 2*(i-u)+u (u = i&1023), partner at
    +1024 (+512 in the leftover block) -> 48 member rows
  - indirect-gather fp32 normalized keys, exact fp32 rescore,
    top-8 of 48 (position packed in low mantissa), softmax,
    indirect-gather value rows, weighted sum.
"""

import os
import sys
import time

_TRN_REPO = "/opt/trn_rl_repo"
if _TRN_REPO not in sys.path:
    sys.path.insert(0, _TRN_REPO)

import numpy as np

import concourse.bass as bass
import concourse.mybir as mybir
import concourse.tile as tile
from concourse import bacc
from concourse.bass import IndirectOffsetOnAxis
from concourse.bass_utils import run_bass_kernel_spmd

F32 = mybir.dt.float32
BF16 = mybir.dt.bfloat16
I32 = mybir.dt.int32
U32 = mybir.dt.uint32
ALU = mybir.AluOpType
ACTF = mybir.ActivationFunctionType
AX = mybir.AxisListType

# ---- problem constants ----
B = 2048
D = 256
N = 100000
TOPK = 8
NCORES = 8
NT = 512
NLOC = 13312              # 26 * 512 per-core shard; 8*13312 = 106496 >= N
NPAD = NLOC * NCORES
M1W = NLOC // 2           # 6656
M2W = NLOC // 4           # 3328
BSLOTS = 14               # cells kept per query after the cross-core merge
NMEMB = BSLOTS * 4        # 4 member keys per cell

_NEG_BIG = -3.0e38


# --------------------------------------------------------------------------
# Dispatch A
# --------------------------------------------------------------------------

def build_dispatch_a(bq=B, nloc=NLOC, dbg=False):
    qtiles = bq // 128
    ntiles = nloc // NT            # 26
    nblk = ntiles // 4             # 6 full blocks (psA+psB) of 2048 keys
    m1w = nloc // 2
    m2w = nloc // 4
    assert ntiles == nblk * 4 + 2  # leftover pair of n-tiles (1024 keys)

    nc = bacc.Bacc("TRN2", target_bir_lowering=False, debug=dbg)
    xT_d = nc.dram_tensor("xT", [2, 128, bq], BF16, kind="ExternalInput").ap()
    kT_d = nc.dram_tensor("kT", [2, 128, nloc], BF16, kind="ExternalInput").ap()
    out_d = nc.dram_tensor("cand", [bq, 8], F32, kind="ExternalOutput").ap()

    with tile.TileContext(nc) as tc:
        with (
            tc.tile_pool(name="const", bufs=1) as constp,
            tc.tile_pool(name="big", bufs=1) as bigp,
            tc.tile_pool(name="sa", bufs=6) as sap,
            tc.tile_pool(name="m1p", bufs=2) as m1p,
            tc.tile_pool(name="m2p", bufs=2) as m2p,
            tc.tile_pool(name="pkp", bufs=2) as pkp,
            tc.tile_pool(name="top", bufs=2) as topp,
            tc.tile_pool(name="ps", bufs=2, space="PSUM") as psp,
        ):
            iota_pack = constp.tile([128, m2w], I32)
            nc.gpsimd.iota(iota_pack[:], pattern=[[1, m2w]], base=0,
                           channel_multiplier=0)
            zero_s = constp.tile([128, 1], I32)
            nc.gpsimd.memset(zero_s[:], 0)

            xt = bigp.tile([128, 2, bq], BF16)
            nc.sync.dma_start(out=xt[:],
                              in_=xT_d[:].rearrange("c p n -> p c n"))
            # kt in per-block chunks so the first matmuls start early
            kt_blocks = []
            for blk in range(nblk + 1):
                w = 2048 if blk < nblk else 1024
                ktb = bigp.tile([128, 2, w], BF16, tag=f"ktb{blk}")
                nc.sync.dma_start(
                    out=ktb[:],
                    in_=kT_d[:, :, blk * 2048:blk * 2048 + w].rearrange(
                        "c p n -> p c n"))
                kt_blocks.append(ktb)

            for qt in range(qtiles):
                q0 = qt * 128
                m1 = m1p.tile([128, m1w], BF16, tag="m1")

                def mm_pair(ps_dst, nt0):
                    # fill [128, 1024] PSUM with scores for 2 n-tiles
                    for half in range(2):
                        dst = ps_dst[:, half * NT:(half + 1) * NT]
                        nti = nt0 + half
                        ktb = kt_blocks[nti // 4]
                        lo = (nti % 4) * NT
                        for c in range(2):
                            nc.tensor.matmul(
                                dst,
                                lhsT=xt[:, c, q0:q0 + 128],
                                rhs=ktb[:, c, lo:lo + NT],
                                start=(c == 0), stop=(c == 1))

                for blk in range(nblk):
                    psA = psp.tile([128, 1024], F32, tag="psA")
                    psB = psp.tile([128, 1024], F32, tag="psB")
                    mm_pair(psA, 4 * blk)
                    mm_pair(psB, 4 * blk + 2)
                    dst = m1[:, blk * 1024:(blk + 1) * 1024]
                    if blk < 1:
                        # DVE-direct: one ACT copy + TT max vs PSUM
                        sA = sap.tile([128, 1024], BF16, tag="sA")
                        nc.scalar.activation(sA[:], psA[:], ACTF.Copy)
                        nc.vector.tensor_tensor(dst, psB[:], sA[:], op=ALU.max)
                    else:
                        # ACT-extract: two ACT copies + bf16 TT max at 2x
                        sA = sap.tile([128, 1024], BF16, tag="sA")
                        sB = sap.tile([128, 1024], BF16, tag="sB")
                        nc.scalar.activation(sA[:], psA[:], ACTF.Copy)
                        nc.scalar.activation(sB[:], psB[:], ACTF.Copy)
                        nc.vector.tensor_tensor(dst, sA[:], sB[:], op=ALU.max)

                # leftover pair (n-tiles 24, 25 -> keys 12288..13311)
                psL = psp.tile([128, 1024], F32, tag="psA")
                mm_pair(psL, 4 * nblk)
                sL = sap.tile([128, 1024], BF16, tag="sA")
                nc.scalar.activation(sL[:], psL[:], ACTF.Copy)
                nc.vector.tensor_tensor(m1[:, nblk * 1024:nblk * 1024 + 512],
                                        sL[:, :512], sL[:, 512:], op=ALU.max)

                # fold2: halves pairing -> m2 [128, 3328]
                m2 = m2p.tile([128, m2w], BF16, tag="m2")
                nc.vector.tensor_tensor(m2[:], m1[:, :m2w], m1[:, m2w:],
                                        op=ALU.max)

                # pack on GPSIMD: bf16 -> fp32 (== bits<<16), OR slot iota
                # pack: Pool converts bf16 -> fp32 (== bits<<16, low 16 bits
                # zero); DVE ORs the 12-bit slot iota into the low mantissa.
                # (Pool cannot do exact int32 arithmetic -- its "int" ops run
                # through the fp32 SIMD and round bit patterns.)
                pk = pkp.tile([128, m2w], F32, tag="pk")
                nc.gpsimd.tensor_copy(pk[:], m2[:])
                nc.vector.scalar_tensor_tensor(
                    pk[:].bitcast(I32), pk[:].bitcast(I32), zero_s[:],
                    iota_pack[:], op0=ALU.bitwise_or, op1=ALU.bitwise_or)

                top = topp.tile([128, 8], F32, tag="top")
                nc.vector.max(out=top[:], in_=pk[:])
                nc.sync.dma_start(out=out_d[q0:q0 + 128, :], in_=top[:])

    nc.compile()
    return nc


# --------------------------------------------------------------------------
# Dispatch B
# --------------------------------------------------------------------------

def build_dispatch_b(bq_slice, nloc=NLOC, npad=NPAD, ncand=NCORES * 8,
                     bslots=BSLOTS, dbg=False):
    qtiles = bq_slice // 128
    m2w = nloc // 4
    nmemb = bslots * 4

    nc = bacc.Bacc("TRN2", target_bir_lowering=False, debug=dbg)
    v_d = nc.dram_tensor("vals", [bq_slice, ncand], F32, kind="ExternalInput").ap()
    x_d = nc.dram_tensor("xn", [bq_slice, D], F32, kind="ExternalInput").ap()
    k_d = nc.dram_tensor("kn", [npad, D], F32, kind="ExternalInput").ap()
    val_d = nc.dram_tensor("values", [npad, D], F32, kind="ExternalInput").ap()
    out_d = nc.dram_tensor("out", [bq_slice, D], F32, kind="ExternalOutput").ap()

    with tile.TileContext(nc) as tc:
        with (
            tc.tile_pool(name="const", bufs=1) as constp,
            tc.tile_pool(name="wp", bufs=2) as wp,
            tc.tile_pool(name="gp", bufs=2) as gp,
        ):
            iota_m_i = constp.tile([128, nmemb], I32)
            nc.gpsimd.iota(iota_m_i[:], pattern=[[1, nmemb]], base=0,
                           channel_multiplier=0)
            iota_m_f = constp.tile([128, nmemb], F32)
            nc.gpsimd.tensor_copy(iota_m_f[:], iota_m_i[:])
            mask64 = constp.tile([128, 1], I32)
            nc.gpsimd.memset(mask64[:], -64)

            for qt in range(qtiles):
                r0, r1 = qt * 128, (qt + 1) * 128

                xn = wp.tile([128, D], F32, tag="xn")
                nc.sync.dma_start(out=xn[:], in_=x_d[r0:r1, :])

                vin = wp.tile([128, ncand], F32, tag="vin")
                nc.sync.dma_start(out=vin[:], in_=v_d[r0:r1, :])

                # --- prune to top-`bslots` cells (tie-free: low 16 bits of
                # every candidate are distinct: core<<12 | j) ---
                t16 = wp.tile([128, 16], F32, tag="t16")
                nc.vector.max(out=t16[:, 0:8], in_=vin[:])
                vrep = wp.tile([128, ncand], F32, tag="vrep")
                nc.vector.match_replace(out=vrep[:], in_to_replace=t16[:, 0:8],
                                        in_values=vin[:], imm_value=_NEG_BIG)
                nc.vector.max(out=t16[:, 8:16], in_=vrep[:])

                # --- decode winners: core, j, j+m2w -> 4 member rows each ---
                # ii [128, 2*bslots] = {j, j+m2w}
                win = wp.tile([128, bslots], I32, tag="win")
                nc.vector.tensor_scalar(win[:], t16[:, :bslots].bitcast(I32),
                                        0x7FFF, None, op0=ALU.bitwise_and)
                corebase = wp.tile([128, bslots], I32, tag="corebase")
                # core*nloc: nloc = 13312 = (1<<13)+(1<<12)+(1<<10)
                core_t = wp.tile([128, bslots], I32, tag="core_t")
                nc.vector.tensor_scalar(core_t[:], win[:], 12, None,
                                        op0=ALU.logical_shift_right)
                cb_t = wp.tile([128, bslots], I32, tag="cb_t")
                nc.vector.tensor_scalar(corebase[:], core_t[:], 13, None,
                                        op0=ALU.logical_shift_left)
                nc.vector.tensor_scalar(cb_t[:], core_t[:], 12, None,
                                        op0=ALU.logical_shift_left)
                nc.vector.tensor_tensor(corebase[:], corebase[:], cb_t[:],
                                        op=ALU.add)
                nc.vector.tensor_scalar(cb_t[:], core_t[:], 10, None,
                                        op0=ALU.logical_shift_left)
                nc.vector.tensor_tensor(corebase[:], corebase[:], cb_t[:],
                                        op=ALU.add)
                jj = wp.tile([128, bslots], I32, tag="jj")
                nc.vector.tensor_scalar(jj[:], win[:], 0xFFF, None,
                                        op0=ALU.bitwise_and)

                ii = wp.tile([128, 2 * bslots], I32, tag="ii")
                nc.vector.tensor_copy(ii[:, :bslots], jj[:])
                nc.vector.tensor_scalar(ii[:, bslots:], jj[:], m2w, None,
                                        op0=ALU.add)
                # u = i & 1023 ; key0 = 2*(i-u) + u ; off = 1024 - 512*(i>=6144)
                uu = wp.tile([128, 2 * bslots], I32, tag="uu")
                nc.vector.tensor_scalar(uu[:], ii[:], 1023, None,
                                        op0=ALU.bitwise_and)
                key0 = wp.tile([128, 2 * bslots], I32, tag="key0")
                nc.vector.tensor_tensor(key0[:], ii[:], uu[:], op=ALU.subtract)
                nc.vector.tensor_scalar(key0[:], key0[:], 1, None,
                                        op0=ALU.logical_shift_left)
                nc.vector.tensor_tensor(key0[:], key0[:], uu[:], op=ALU.add)
                # partner offset: 1024, except 512 in the leftover block
                # (i >= 6144) -> key1 = key0 + 1024 - (ge << 9)
                ge = wp.tile([128, 2 * bslots], I32, tag="ge")
                nc.vector.tensor_scalar(ge[:], ii[:], 3 * 2048, None,
                                        op0=ALU.is_ge)
                ge512 = wp.tile([128, 2 * bslots], I32, tag="ge512")
                nc.vector.tensor_scalar(ge512[:], ge[:], 9, None,
                                        op0=ALU.logical_shift_left)

                # rows [128, nmemb]: layout [:, 0:2b] = key0(ii)+corebase,
                # [:, 2b:4b] = that + off(ii)
                rows_i = wp.tile([128, nmemb], I32, tag="rowsi")
                cb2 = wp.tile([128, 2 * bslots], I32, tag="cb2")
                nc.vector.tensor_copy(cb2[:, :bslots], corebase[:])
                nc.vector.tensor_copy(cb2[:, bslots:], corebase[:])
                nc.vector.tensor_tensor(rows_i[:, :2 * bslots], key0[:], cb2[:],
                                        op=ALU.add)
                nc.vector.tensor_scalar(rows_i[:, 2 * bslots:],
                                        rows_i[:, :2 * bslots], 1024, None,
                                        op0=ALU.add)
                nc.vector.tensor_tensor(rows_i[:, 2 * bslots:],
                                        rows_i[:, 2 * bslots:], ge512[:],
                                        op=ALU.subtract)
                rows_f = wp.tile([128, nmemb], F32, tag="rowsf")
                nc.vector.tensor_copy(rows_f[:], rows_i[:])

                # --- gather member rows + exact fp32 rescore (2 chunks) ---
                half = nmemb // 2
                sco = wp.tile([128, nmemb], F32, tag="sco")
                for m in range(2):
                    g = gp.tile([128, half, D], F32, tag="g")
                    for s in range(half):
                        nc.gpsimd.indirect_dma_start(
                            out=g[:, s, :], out_offset=None, in_=k_d[:],
                            in_offset=IndirectOffsetOnAxis(
                                ap=rows_i[:, m * half + s:m * half + s + 1],
                                axis=0))
                    prod = gp.tile([128, half, D], F32, tag="prod")
                    xb = xn[:].unsqueeze(1).to_broadcast([128, half, D])
                    nc.vector.tensor_tensor(prod[:], g[:], xb, op=ALU.mult)
                    nc.vector.tensor_reduce(sco[:, m * half:(m + 1) * half],
                                            prod[:], axis=AX.X, op=ALU.add)

                # --- pack member position into low mantissa, top-8 of 48 ---
                scp = wp.tile([128, nmemb], F32, tag="scp")
                nc.vector.scalar_tensor_tensor(
                    scp[:].bitcast(I32), sco[:].bitcast(I32),
                    mask64[:], iota_m_i[:],
                    op0=ALU.bitwise_and, op1=ALU.bitwise_or)
                top8 = wp.tile([128, 8], F32, tag="top8")
                nc.vector.max(out=top8[:], in_=scp[:])
                pos8 = wp.tile([128, 8], I32, tag="pos8")
                nc.vector.tensor_scalar(pos8[:], top8[:].bitcast(I32), 63, None,
                                        op0=ALU.bitwise_and)
                pos8f = wp.tile([128, 8], F32, tag="pos8f")
                nc.vector.tensor_copy(pos8f[:], pos8[:])
                sc8 = wp.tile([128, 8], F32, tag="sc8")
                nc.vector.tensor_scalar(sc8[:].bitcast(I32),
                                        top8[:].bitcast(I32), -64, None,
                                        op0=ALU.bitwise_and)

                # --- softmax over the 8 (top8[:,0] is the max) ---
                sh = wp.tile([128, 8], F32, tag="sh")
                nc.vector.tensor_tensor(sh[:], sc8[:],
                                        sc8[:, 0:1].to_broadcast([128, 8]),
                                        op=ALU.subtract)
                ex = wp.tile([128, 8], F32, tag="ex")
                nc.scalar.activation(ex[:], sh[:], ACTF.Exp)
                es = wp.tile([128, 1], F32, tag="es")
                nc.vector.tensor_reduce(es[:], ex[:], axis=AX.X, op=ALU.add)
                esr = wp.tile([128, 1], F32, tag="esr")
                nc.vector.reciprocal(esr[:], es[:])
                wgt = wp.tile([128, 8], F32, tag="wgt")
                nc.vector.tensor_tensor(wgt[:], ex[:],
                                        esr[:].to_broadcast([128, 8]),
                                        op=ALU.mult)

                # --- winner rows via one-hot over member index ---
                winr = wp.tile([128, 8], F32, tag="winr")
                ohm = wp.tile([128, nmemb], F32, tag="ohm")
                for w in range(8):
                    nc.vector.tensor_tensor(
                        ohm[:], iota_m_f[:],
                        pos8f[:, w:w + 1].to_broadcast([128, nmemb]),
                        op=ALU.is_equal)
                    nc.vector.tensor_tensor(ohm[:], ohm[:], rows_f[:],
                                            op=ALU.mult)
                    nc.vector.tensor_reduce(winr[:, w:w + 1], ohm[:], axis=AX.X,
                                            op=ALU.add)
                winr_i = wp.tile([128, 8], I32, tag="winri")
                nc.vector.tensor_copy(winr_i[:], winr[:])

                # --- gather value rows, weighted sum ---
                vg = gp.tile([128, 8, D], F32, tag="vg")
                for k in range(8):
                    nc.gpsimd.indirect_dma_start(
                        out=vg[:, k, :], out_offset=None, in_=val_d[:],
                        in_offset=IndirectOffsetOnAxis(ap=winr_i[:, k:k + 1],
                                                       axis=0))
                vw = gp.tile([128, 8, D], F32, tag="vw")
                nc.vector.tensor_tensor(
                    vw[:], vg[:],
                    wgt[:].unsqueeze(2).to_broadcast([128, 8, D]), op=ALU.mult)
                ot = wp.tile([128, D], F32, tag="ot")
                nc.vector.tensor_reduce(ot[:], vw[:].rearrange("p k d -> p d k"),
                                        axis=AX.X, op=ALU.add)
                nc.sync.dma_start(out=out_d[r0:r1, :], in_=ot[:])

    nc.compile()
    return nc


# --------------------------------------------------------------------------
# Host orchestration
# --------------------------------------------------------------------------

_CACHE = {}
TRACE = False
CAPTURE = False
last_results = None
last_capture = None


def _get_programs():
    if "A" not in _CACHE:
        _CACHE["A"] = build_dispatch_a()
    if "B" not in _CACHE:
        _CACHE["B"] = build_dispatch_b(B // NCORES)
    return _CACHE["A"], _CACHE["B"]


def _normalize_rows(a):
    n = np.sqrt(np.sum(a.astype(np.float64) ** 2, axis=1, keepdims=True))
    n = np.maximum(n, 1e-12)
    return (a / n).astype(np.float32)


def kernel(x, keys, values, top_k):
    assert int(top_k) == TOPK
    import ml_dtypes

    x = np.ascontiguousarray(np.asarray(x, dtype=np.float32))
    keys = np.asarray(keys, dtype=np.float32)
    values = np.asarray(values, dtype=np.float32)
    assert x.shape == (B, D) and keys.shape == (N, D) and values.shape == (N, D)

    xn = _normalize_rows(x)                       # [B, D] fp32
    kn_pad = np.zeros((NPAD, D), dtype=np.float32)
    kn_pad[:N] = _normalize_rows(keys)
    values_pad = np.zeros((NPAD, D), dtype=np.float32)
    values_pad[:N] = values

    # bf16 transposed views for dispatch A: [2, 128, cols] (c-half, partition)
    xT = np.ascontiguousarray(xn.T).astype(ml_dtypes.bfloat16).reshape(2, 128, B)
    kT_full = np.ascontiguousarray(kn_pad.T).astype(ml_dtypes.bfloat16)

    nc_a, nc_b = _get_programs()
    core_ids = list(range(NCORES))
    tkw = {"trace": True} if TRACE else {}

    in_maps_a = [
        {"xT": xT,
         "kT": np.ascontiguousarray(
             kT_full[:, c * NLOC:(c + 1) * NLOC]).reshape(2, 128, NLOC)}
        for c in range(NCORES)
    ]
    t0 = time.perf_counter()
    res_a = run_bass_kernel_spmd(nc_a, in_maps_a, core_ids, **tkw)
    t1 = time.perf_counter()
    cand = np.concatenate([res_a.results[c]["cand"] for c in range(NCORES)],
                          axis=1)  # [B, 64]
    # OR core id into bits 12:15 of each packed candidate
    ci = cand.view(np.uint32)
    ci |= (np.arange(NCORES, dtype=np.uint32).repeat(8)[None, :] << 12)

    bs = B // NCORES
    in_maps_b = [
        {
            "vals": np.ascontiguousarray(cand[c * bs:(c + 1) * bs]),
            "xn": np.ascontiguousarray(xn[c * bs:(c + 1) * bs]),
            "kn": kn_pad,
            "values": values_pad,
        }
        for c in range(NCORES)
    ]
    t2 = time.perf_counter()
    res_b = run_bass_kernel_spmd(nc_b, in_maps_b, core_ids, **tkw)
    t3 = time.perf_counter()
    out = np.concatenate([res_b.results[c]["out"] for c in range(NCORES)],
                         axis=0)
    kernel.last_walltimes = (t1 - t0, t3 - t2)
    if TRACE:
        global last_results
        last_results = (res_a, res_b)
    if CAPTURE:
        global last_capture
        last_capture = (nc_a, in_maps_a, nc_b, in_maps_b)
    return out.astype(np.float32)


# revision 42
# speedup vs baseline: 28.3107x; 28.3107x over previous
"""Distributed cosine-similarity kNN retrieval (EpisodicSDM) on 8 Trainium2 cores.

Reference:
    x_n = normalize(x); k_n = normalize(keys)
    scores = x_n @ k_n.T                [B, N]
    top_vals, top_idx = top_k(scores, 8)
    out = sum_k softmax(top_vals)_k * values[top_idx_k]

Host prep (pure data movement / O(N*D) scaling, no HW time):
  - normalize x and keys in fp32; pad keys/values to NPAD rows
  - pre-transpose to bf16: xT [D, B], kT [D, NLOC] per-core shard

Dispatch A (keys sharded along N, all queries on every core):
  - scores S = xT.T @ kT in bf16 (fp32 PSUM), 26 n-tiles of 512 per
    128-query tile, grouped into 6 psA/psB block pairs + 1 leftover
  - fold1 -> m1 bf16 [128, 6656]: 3 blocks DVE-direct (ACT copies psA
    to SBUF bf16, DVE TT max vs psB in PSUM), 3 blocks ACT-extracted
    (both copies on ACT, DVE TT max in bf16 at 2x), leftover self-max
  - fold2 -> m2 bf16 [128, 3328] (DVE TT bf16 2x, halves pairing)
  - pack on GPSIMD: numeric bf16->fp32 convert (== bits<<16), then OR
    a 12-bit slot iota into the low mantissa -> tie-free packed fp32
  - DVE max8 -> per-core top-8 packed cells [B, 8]

Host glue: concat candidates [B, 64]; OR core id into bits 12:15.

Dispatch B (queries sharded, 256 per core; full key/value tables):
  - top-12 of 64 via max8 + match_replace + max8 (tie-free: low 16
    bits of every candidate are distinct)
  - decode rows arithmetically: j = v&0xFFF, core = (v>>12)&7,
    i in {j, j+3328}; key0 = 2*(i-u)+u (u = i&1023), partner at
    +1024 (+512 in the leftover block) -> 48 member rows
  - indirect-gather fp32 normalized keys, exact fp32 rescore,
    top-8 of 48 (position packed in low mantissa), softmax,
    indirect-gather value rows, weighted sum.
"""

import os
import sys
import time

_TRN_REPO = "/opt/trn_rl_repo"
if _TRN_REPO not in sys.path:
    sys.path.insert(0, _TRN_REPO)

import numpy as np

import concourse.bass as bass
import concourse.mybir as mybir
import concourse.tile as tile
from concourse import bacc
from concourse.bass import IndirectOffsetOnAxis
from concourse.bass_utils import run_bass_kernel_spmd

F32 = mybir.dt.float32
BF16 = mybir.dt.bfloat16
I32 = mybir.dt.int32
U32 = mybir.dt.uint32
ALU = mybir.AluOpType
ACTF = mybir.ActivationFunctionType
AX = mybir.AxisListType

# ---- problem constants ----
B = 2048
D = 256
N = 100000
TOPK = 8
NCORES = 8
NT = 512
NLOC = 13312              # 26 * 512 per-core shard; 8*13312 = 106496 >= N
NPAD = NLOC * NCORES
M1W = NLOC // 2           # 6656
M2W = NLOC // 4           # 3328
BSLOTS = 14               # cells kept per query after the cross-core merge
NMEMB = BSLOTS * 4        # 4 member keys per cell

_NEG_BIG = -3.0e38


# --------------------------------------------------------------------------
# Dispatch A
# --------------------------------------------------------------------------

def build_dispatch_a(bq=B, nloc=NLOC, dbg=False, reps=1):
    qtiles = bq // 128
    ntiles = nloc // NT            # 26
    nblk = ntiles // 4             # 6 full blocks (psA+psB) of 2048 keys
    m1w = nloc // 2
    m2w = nloc // 4
    assert ntiles == nblk * 4 + 2  # leftover pair of n-tiles (1024 keys)

    nc = bacc.Bacc("TRN2", target_bir_lowering=False, debug=dbg)
    xT_d = nc.dram_tensor("xT", [2, 128, bq], BF16, kind="ExternalInput").ap()
    kT_d = nc.dram_tensor("kT", [2, 128, nloc], BF16, kind="ExternalInput").ap()
    out_d = nc.dram_tensor("cand", [bq, 8], F32, kind="ExternalOutput").ap()

    with tile.TileContext(nc) as tc:
        with (
            tc.tile_pool(name="const", bufs=1) as constp,
            tc.tile_pool(name="big", bufs=1) as bigp,
            tc.tile_pool(name="sa", bufs=6) as sap,
            tc.tile_pool(name="m1p", bufs=2) as m1p,
            tc.tile_pool(name="m2p", bufs=2) as m2p,
            tc.tile_pool(name="pkp", bufs=2) as pkp,
            tc.tile_pool(name="top", bufs=2) as topp,
            tc.tile_pool(name="ps", bufs=2, space="PSUM") as psp,
        ):
            iota_pack = constp.tile([128, m2w], I32)
            nc.gpsimd.iota(iota_pack[:], pattern=[[1, m2w]], base=0,
                           channel_multiplier=0)
            zero_s = constp.tile([128, 1], I32)
            nc.gpsimd.memset(zero_s[:], 0)

            xt = bigp.tile([128, 2, bq], BF16)
            nc.sync.dma_start(out=xt[:],
                              in_=xT_d[:].rearrange("c p n -> p c n"))
            # kt in per-block chunks so the first matmuls start early
            kt_blocks = []
            for blk in range(nblk + 1):
                w = 2048 if blk < nblk else 1024
                ktb = bigp.tile([128, 2, w], BF16, tag=f"ktb{blk}")
                nc.sync.dma_start(
                    out=ktb[:],
                    in_=kT_d[:, :, blk * 2048:blk * 2048 + w].rearrange(
                        "c p n -> p c n"))
                kt_blocks.append(ktb)

            for qt in range(qtiles * reps):
                qt = qt % qtiles
                q0 = qt * 128
                m1 = m1p.tile([128, m1w], BF16, tag="m1")

                def mm_pair(ps_dst, nt0):
                    # fill [128, 1024] PSUM with scores for 2 n-tiles
                    for half in range(2):
                        dst = ps_dst[:, half * NT:(half + 1) * NT]
                        nti = nt0 + half
                        ktb = kt_blocks[nti // 4]
                        lo = (nti % 4) * NT
                        for c in range(2):
                            nc.tensor.matmul(
                                dst,
                                lhsT=xt[:, c, q0:q0 + 128],
                                rhs=ktb[:, c, lo:lo + NT],
                                start=(c == 0), stop=(c == 1))

                for blk in range(nblk):
                    psA = psp.tile([128, 1024], F32, tag="psA")
                    psB = psp.tile([128, 1024], F32, tag="psB")
                    mm_pair(psA, 4 * blk)
                    mm_pair(psB, 4 * blk + 2)
                    dst = m1[:, blk * 1024:(blk + 1) * 1024]
                    if blk < 1:
                        # DVE-direct: one ACT copy + TT max vs PSUM
                        sA = sap.tile([128, 1024], BF16, tag="sA")
                        nc.scalar.activation(sA[:], psA[:], ACTF.Copy)
                        nc.vector.tensor_tensor(dst, psB[:], sA[:], op=ALU.max)
                    else:
                        # ACT-extract: two ACT copies + bf16 TT max at 2x
                        sA = sap.tile([128, 1024], BF16, tag="sA")
                        sB = sap.tile([128, 1024], BF16, tag="sB")
                        nc.scalar.activation(sA[:], psA[:], ACTF.Copy)
                        nc.scalar.activation(sB[:], psB[:], ACTF.Copy)
                        nc.vector.tensor_tensor(dst, sA[:], sB[:], op=ALU.max)

                # leftover pair (n-tiles 24, 25 -> keys 12288..13311)
                psL = psp.tile([128, 1024], F32, tag="psA")
                mm_pair(psL, 4 * nblk)
                sL = sap.tile([128, 1024], BF16, tag="sA")
                nc.scalar.activation(sL[:], psL[:], ACTF.Copy)
                nc.vector.tensor_tensor(m1[:, nblk * 1024:nblk * 1024 + 512],
                                        sL[:, :512], sL[:, 512:], op=ALU.max)

                # fold2: halves pairing -> m2 [128, 3328]
                m2 = m2p.tile([128, m2w], BF16, tag="m2")
                nc.vector.tensor_tensor(m2[:], m1[:, :m2w], m1[:, m2w:],
                                        op=ALU.max)

                # pack on GPSIMD: bf16 -> fp32 (== bits<<16), OR slot iota
                # pack: Pool converts bf16 -> fp32 (== bits<<16, low 16 bits
                # zero); DVE ORs the 12-bit slot iota into the low mantissa.
                # (Pool cannot do exact int32 arithmetic -- its "int" ops run
                # through the fp32 SIMD and round bit patterns.)
                pk = pkp.tile([128, m2w], F32, tag="pk")
                nc.gpsimd.tensor_copy(pk[:], m2[:])
                nc.vector.scalar_tensor_tensor(
                    pk[:].bitcast(I32), pk[:].bitcast(I32), zero_s[:],
                    iota_pack[:], op0=ALU.bitwise_or, op1=ALU.bitwise_or)

                top = topp.tile([128, 8], F32, tag="top")
                nc.vector.max(out=top[:], in_=pk[:])
                nc.sync.dma_start(out=out_d[q0:q0 + 128, :], in_=top[:])

    nc.compile()
    return nc


# --------------------------------------------------------------------------
# Dispatch B
# --------------------------------------------------------------------------

def build_dispatch_b(bq_slice, nloc=NLOC, npad=NPAD, ncand=NCORES * 8,
                     bslots=BSLOTS, dbg=False, reps=1):
    qtiles = bq_slice // 128
    m2w = nloc // 4
    nmemb = bslots * 4

    nc = bacc.Bacc("TRN2", target_bir_lowering=False, debug=dbg)
    v_d = nc.dram_tensor("vals", [bq_slice, ncand], F32, kind="ExternalInput").ap()
    x_d = nc.dram_tensor("xn", [bq_slice, D], F32, kind="ExternalInput").ap()
    k_d = nc.dram_tensor("kn", [npad, D], F32, kind="ExternalInput").ap()
    val_d = nc.dram_tensor("values", [npad, D], F32, kind="ExternalInput").ap()
    out_d = nc.dram_tensor("out", [bq_slice, D], F32, kind="ExternalOutput").ap()

    with tile.TileContext(nc) as tc:
        with (
            tc.tile_pool(name="const", bufs=1) as constp,
            tc.tile_pool(name="wp", bufs=2) as wp,
            tc.tile_pool(name="gp", bufs=2) as gp,
        ):
            iota_m_i = constp.tile([128, nmemb], I32)
            nc.gpsimd.iota(iota_m_i[:], pattern=[[1, nmemb]], base=0,
                           channel_multiplier=0)
            iota_m_f = constp.tile([128, nmemb], F32)
            nc.gpsimd.tensor_copy(iota_m_f[:], iota_m_i[:])
            mask64 = constp.tile([128, 1], I32)
            nc.gpsimd.memset(mask64[:], -64)

            for qt in range(qtiles * reps):
                qt = qt % qtiles
                r0, r1 = qt * 128, (qt + 1) * 128

                xn = wp.tile([128, D], F32, tag="xn")
                nc.sync.dma_start(out=xn[:], in_=x_d[r0:r1, :])

                vin = wp.tile([128, ncand], F32, tag="vin")
                nc.sync.dma_start(out=vin[:], in_=v_d[r0:r1, :])

                # --- prune to top-`bslots` cells (tie-free: low 16 bits of
                # every candidate are distinct: core<<12 | j) ---
                t16 = wp.tile([128, 16], F32, tag="t16")
                nc.vector.max(out=t16[:, 0:8], in_=vin[:])
                vrep = wp.tile([128, ncand], F32, tag="vrep")
                nc.vector.match_replace(out=vrep[:], in_to_replace=t16[:, 0:8],
                                        in_values=vin[:], imm_value=_NEG_BIG)
                nc.vector.max(out=t16[:, 8:16], in_=vrep[:])

                # --- decode winners: core, j, j+m2w -> 4 member rows each ---
                # ii [128, 2*bslots] = {j, j+m2w}
                win = wp.tile([128, bslots], I32, tag="win")
                nc.vector.tensor_scalar(win[:], t16[:, :bslots].bitcast(I32),
                                        0x7FFF, None, op0=ALU.bitwise_and)
                corebase = wp.tile([128, bslots], I32, tag="corebase")
                # core*nloc: nloc = 13312 = (1<<13)+(1<<12)+(1<<10)
                core_t = wp.tile([128, bslots], I32, tag="core_t")
                nc.vector.tensor_scalar(core_t[:], win[:], 12, None,
                                        op0=ALU.logical_shift_right)
                cb_t = wp.tile([128, bslots], I32, tag="cb_t")
                nc.vector.tensor_scalar(corebase[:], core_t[:], 13, None,
                                        op0=ALU.logical_shift_left)
                nc.vector.tensor_scalar(cb_t[:], core_t[:], 12, None,
                                        op0=ALU.logical_shift_left)
                nc.vector.tensor_tensor(corebase[:], corebase[:], cb_t[:],
                                        op=ALU.add)
                nc.vector.tensor_scalar(cb_t[:], core_t[:], 10, None,
                                        op0=ALU.logical_shift_left)
                nc.vector.tensor_tensor(corebase[:], corebase[:], cb_t[:],
                                        op=ALU.add)
                jj = wp.tile([128, bslots], I32, tag="jj")
                nc.vector.tensor_scalar(jj[:], win[:], 0xFFF, None,
                                        op0=ALU.bitwise_and)

                ii = wp.tile([128, 2 * bslots], I32, tag="ii")
                nc.vector.tensor_copy(ii[:, :bslots], jj[:])
                nc.vector.tensor_scalar(ii[:, bslots:], jj[:], m2w, None,
                                        op0=ALU.add)
                # u = i & 1023 ; key0 = 2*(i-u) + u ; off = 1024 - 512*(i>=6144)
                uu = wp.tile([128, 2 * bslots], I32, tag="uu")
                nc.vector.tensor_scalar(uu[:], ii[:], 1023, None,
                                        op0=ALU.bitwise_and)
                key0 = wp.tile([128, 2 * bslots], I32, tag="key0")
                nc.vector.tensor_tensor(key0[:], ii[:], uu[:], op=ALU.subtract)
                nc.vector.tensor_scalar(key0[:], key0[:], 1, None,
                                        op0=ALU.logical_shift_left)
                nc.vector.tensor_tensor(key0[:], key0[:], uu[:], op=ALU.add)
                # partner offset: 1024, except 512 in the leftover block
                # (i >= 6144) -> key1 = key0 + 1024 - (ge << 9)
                ge = wp.tile([128, 2 * bslots], I32, tag="ge")
                nc.vector.tensor_scalar(ge[:], ii[:], 3 * 2048, None,
                                        op0=ALU.is_ge)
                ge512 = wp.tile([128, 2 * bslots], I32, tag="ge512")
                nc.vector.tensor_scalar(ge512[:], ge[:], 9, None,
                                        op0=ALU.logical_shift_left)

                # rows [128, nmemb]: layout [:, 0:2b] = key0(ii)+corebase,
                # [:, 2b:4b] = that + off(ii)
                rows_i = wp.tile([128, nmemb], I32, tag="rowsi")
                cb2 = wp.tile([128, 2 * bslots], I32, tag="cb2")
                nc.vector.tensor_copy(cb2[:, :bslots], corebase[:])
                nc.vector.tensor_copy(cb2[:, bslots:], corebase[:])
                nc.vector.tensor_tensor(rows_i[:, :2 * bslots], key0[:], cb2[:],
                                        op=ALU.add)
                nc.vector.tensor_scalar(rows_i[:, 2 * bslots:],
                                        rows_i[:, :2 * bslots], 1024, None,
                                        op0=ALU.add)
                nc.vector.tensor_tensor(rows_i[:, 2 * bslots:],
                                        rows_i[:, 2 * bslots:], ge512[:],
                                        op=ALU.subtract)
                rows_f = wp.tile([128, nmemb], F32, tag="rowsf")
                nc.vector.tensor_copy(rows_f[:], rows_i[:])

                # --- gather member rows + exact fp32 rescore (2 chunks) ---
                half = nmemb // 2
                sco = wp.tile([128, nmemb], F32, tag="sco")
                for m in range(2):
                    g = gp.tile([128, half, D], F32, tag="g")
                    for s in range(half):
                        nc.gpsimd.indirect_dma_start(
                            out=g[:, s, :], out_offset=None, in_=k_d[:],
                            in_offset=IndirectOffsetOnAxis(
                                ap=rows_i[:, m * half + s:m * half + s + 1],
                                axis=0))
                    prod = gp.tile([128, half, D], F32, tag="prod")
                    xb = xn[:].unsqueeze(1).to_broadcast([128, half, D])
                    nc.vector.tensor_tensor(prod[:], g[:], xb, op=ALU.mult)
                    nc.vector.tensor_reduce(sco[:, m * half:(m + 1) * half],
                                            prod[:], axis=AX.X, op=ALU.add)

                # --- pack member position into low mantissa, top-8 of 48 ---
                scp = wp.tile([128, nmemb], F32, tag="scp")
                nc.vector.scalar_tensor_tensor(
                    scp[:].bitcast(I32), sco[:].bitcast(I32),
                    mask64[:], iota_m_i[:],
                    op0=ALU.bitwise_and, op1=ALU.bitwise_or)
                top8 = wp.tile([128, 8], F32, tag="top8")
                nc.vector.max(out=top8[:], in_=scp[:])
                pos8 = wp.tile([128, 8], I32, tag="pos8")
                nc.vector.tensor_scalar(pos8[:], top8[:].bitcast(I32), 63, None,
                                        op0=ALU.bitwise_and)
                pos8f = wp.tile([128, 8], F32, tag="pos8f")
                nc.vector.tensor_copy(pos8f[:], pos8[:])
                sc8 = wp.tile([128, 8], F32, tag="sc8")
                nc.vector.tensor_scalar(sc8[:].bitcast(I32),
                                        top8[:].bitcast(I32), -64, None,
                                        op0=ALU.bitwise_and)

                # --- softmax over the 8 (top8[:,0] is the max) ---
                sh = wp.tile([128, 8], F32, tag="sh")
                nc.vector.tensor_tensor(sh[:], sc8[:],
                                        sc8[:, 0:1].to_broadcast([128, 8]),
                                        op=ALU.subtract)
                ex = wp.tile([128, 8], F32, tag="ex")
                nc.scalar.activation(ex[:], sh[:], ACTF.Exp)
                es = wp.tile([128, 1], F32, tag="es")
                nc.vector.tensor_reduce(es[:], ex[:], axis=AX.X, op=ALU.add)
                esr = wp.tile([128, 1], F32, tag="esr")
                nc.vector.reciprocal(esr[:], es[:])
                wgt = wp.tile([128, 8], F32, tag="wgt")
                nc.vector.tensor_tensor(wgt[:], ex[:],
                                        esr[:].to_broadcast([128, 8]),
                                        op=ALU.mult)

                # --- winner rows via one-hot over member index ---
                winr = wp.tile([128, 8], F32, tag="winr")
                ohm = wp.tile([128, nmemb], F32, tag="ohm")
                for w in range(8):
                    nc.vector.tensor_tensor(
                        ohm[:], iota_m_f[:],
                        pos8f[:, w:w + 1].to_broadcast([128, nmemb]),
                        op=ALU.is_equal)
                    nc.vector.tensor_tensor(ohm[:], ohm[:], rows_f[:],
                                            op=ALU.mult)
                    nc.vector.tensor_reduce(winr[:, w:w + 1], ohm[:], axis=AX.X,
                                            op=ALU.add)
                winr_i = wp.tile([128, 8], I32, tag="winri")
                nc.vector.tensor_copy(winr_i[:], winr[:])

                # --- gather value rows, weighted sum ---
                vg = gp.tile([128, 8, D], F32, tag="vg")
                for k in range(8):
                    nc.gpsimd.indirect_dma_start(
                        out=vg[:, k, :], out_offset=None, in_=val_d[:],
                        in_offset=IndirectOffsetOnAxis(ap=winr_i[:, k:k + 1],
                                                       axis=0))
                vw = gp.tile([128, 8, D], F32, tag="vw")
                nc.vector.tensor_tensor(
                    vw[:], vg[:],
                    wgt[:].unsqueeze(2).to_broadcast([128, 8, D]), op=ALU.mult)
                ot = wp.tile([128, D], F32, tag="ot")
                nc.vector.tensor_reduce(ot[:], vw[:].rearrange("p k d -> p d k"),
                                        axis=AX.X, op=ALU.add)
                nc.sync.dma_start(out=out_d[r0:r1, :], in_=ot[:])

    nc.compile()
    return nc


# --------------------------------------------------------------------------
# Host orchestration
# --------------------------------------------------------------------------

_CACHE = {}
TRACE = False
CAPTURE = False
last_results = None
last_capture = None


def _get_programs():
    if "A" not in _CACHE:
        _CACHE["A"] = build_dispatch_a()
    if "B" not in _CACHE:
        _CACHE["B"] = build_dispatch_b(B // NCORES)
    return _CACHE["A"], _CACHE["B"]


def _normalize_rows(a):
    n = np.sqrt(np.sum(a.astype(np.float64) ** 2, axis=1, keepdims=True))
    n = np.maximum(n, 1e-12)
    return (a / n).astype(np.float32)


def kernel(x, keys, values, top_k):
    assert int(top_k) == TOPK
    import ml_dtypes

    x = np.ascontiguousarray(np.asarray(x, dtype=np.float32))
    keys = np.asarray(keys, dtype=np.float32)
    values = np.asarray(values, dtype=np.float32)
    assert x.shape == (B, D) and keys.shape == (N, D) and values.shape == (N, D)

    xn = _normalize_rows(x)                       # [B, D] fp32
    kn_pad = np.zeros((NPAD, D), dtype=np.float32)
    kn_pad[:N] = _normalize_rows(keys)
    values_pad = np.zeros((NPAD, D), dtype=np.float32)
    values_pad[:N] = values

    # bf16 transposed views for dispatch A: [2, 128, cols] (c-half, partition)
    xT = np.ascontiguousarray(xn.T).astype(ml_dtypes.bfloat16).reshape(2, 128, B)
    kT_full = np.ascontiguousarray(kn_pad.T).astype(ml_dtypes.bfloat16)

    nc_a, nc_b = _get_programs()
    core_ids = list(range(NCORES))
    tkw = {"trace": True} if TRACE else {}

    in_maps_a = [
        {"xT": xT,
         "kT": np.ascontiguousarray(
             kT_full[:, c * NLOC:(c + 1) * NLOC]).reshape(2, 128, NLOC)}
        for c in range(NCORES)
    ]
    t0 = time.perf_counter()
    res_a = run_bass_kernel_spmd(nc_a, in_maps_a, core_ids, **tkw)
    t1 = time.perf_counter()
    cand = np.concatenate([res_a.results[c]["cand"] for c in range(NCORES)],
                          axis=1)  # [B, 64]
    # OR core id into bits 12:15 of each packed candidate
    ci = cand.view(np.uint32)
    ci |= (np.arange(NCORES, dtype=np.uint32).repeat(8)[None, :] << 12)

    bs = B // NCORES
    in_maps_b = [
        {
            "vals": np.ascontiguousarray(cand[c * bs:(c + 1) * bs]),
            "xn": np.ascontiguousarray(xn[c * bs:(c + 1) * bs]),
            "kn": kn_pad,
            "values": values_pad,
        }
        for c in range(NCORES)
    ]
    t2 = time.perf_counter()
    res_b = run_bass_kernel_spmd(nc_b, in_maps_b, core_ids, **tkw)
    t3 = time.perf_counter()
    out = np.concatenate([res_b.results[c]["out"] for c in range(NCORES)],
                         axis=0)
    kernel.last_walltimes = (t1 - t0, t3 - t2)
    if TRACE:
        global last_results
        last_results = (res_a, res_b)
    if CAPTURE:
        global last_capture
        last_capture = (nc_a, in_maps_a, nc_b, in_maps_b)
    return out.astype(np.float32)
